# revision 14
# baseline (speedup 1.0000x reference)
"""Multi-head attention on 8 Trainium2 NeuronCores (Bass/Tile).

Problem: B=4, T=2048, DIM=2048, H=16 heads, dk=dv=64.
  q = Q@Wq, k = K@Wk, v = V@Wv  (per head slices)
  out = softmax(q k^T / sqrt(dk)) v @ Wo

Sharding: data-parallel over batch (4) x query-row halves (2) = 8 cores.
Core (b, s) computes output rows [s*1024:(s+1)*1024] of batch b.
Each core projects k/v for its OWN T-half; the pair exchanges projected
k/v via a 2-rank AllGather (SCHEME_C). Attention + output projection are
core-local. With SCHEME_C=False each core recomputes the partner's k/v
projections instead (no collective).

Device layouts (bf16 compute, fp32 PSUM accumulation):
  xqT/xkT/xvT [D, TQ] = host-transposed input halves (D = contraction dim
    on partitions), wq/wk/wv [D, QK], wo [QK, D] natural (lhsT-ready)
  kT [QK, T]: head h rows 64h..64h+63 -> S^T matmul lhsT
  vaug [T, H, 65]: per head 64 v-cols + ones column (-> softmax row sums)
  S^T tile [Tk-chunk 128, Tq 512] = kT-chunk.T @ qT block (K=dk=64)
  P^T = exp(S^T/8)  (scores bounded ~+-5 -> no max-subtraction pass)
  aoT_aug [65, Tq] per head = vaug.T @ P^T accumulated over Tk chunks;
    row 64 = denominators l; rows/l via DRAM-bounce broadcast of 1/l
  out rows = aoT.T @ Wo accumulated over QK chunks.
"""

import os

import ml_dtypes
import numpy as np

import concourse.bass as bass
from concourse import bacc
import concourse.mybir as mybir
import concourse.tile as tile
from concourse.bass_utils import run_bass_kernel_spmd

BF16 = ml_dtypes.bfloat16
BF = mybir.dt.bfloat16
FP32 = mybir.dt.float32

B = 4
T = 2048
D = 2048
H = 16
DKH = 64
QK = H * DKH   # 1024
TQ = T // 2    # per-core query rows / local T half
KD = D // 128  # 16 contraction chunks for projections
NCORES = 8
SCHEME_C = False  # 2-rank AllGather measured slower than recomputing partner k/v

KN = 128 * (QK // 128) * TQ   # bf16 elems of one kT half
VN = 128 * (TQ // 128) * QK   # bf16 elems of one v half
CCN = KN + VN

LAST = None  # BassKernelResults of the most recent run (for test harness)

_cache = {}


def _install_ntff_shim():
    """Provide antenv.axon_hooks + disable artifact upload so that
    run_bass_kernel_spmd(trace=True) can profile under axon in this image."""
    import sys
    import types

    try:
        import antenv.axon_hooks  # noqa: F401
    except ImportError:
        import antenv
        mod = types.ModuleType("antenv.axon_hooks")
        _h = [None]
        mod.set_axon_ntff_profile_hook = lambda h: _h.__setitem__(0, h)
        mod.get_axon_ntff_profile_hook = lambda: _h[0]
        sys.modules["antenv.axon_hooks"] = mod
        antenv.axon_hooks = mod
        try:
            from trn_agent_boot.trn_boot import _ntff_profile_via_ctypes
            mod.set_axon_ntff_profile_hook(
                _ntff_profile_via_ctypes("/opt/axon/libaxon_pjrt.so"))
        except Exception as e:
            print(f"ntff hook registration failed: {e}")
    try:
        import concourse.bass_utils as bu
        bu.upload_artifacts = lambda tmpdir: f"local:{tmpdir}"
    except Exception:
        pass


def _emit(tc, xqT, xkT, xvT, wq, wk, wv, wo, out, cc_in, cc_out):
    nc = tc.nc
    exp_f = mybir.ActivationFunctionType.Exp
    n_halves = 1 if SCHEME_C else 2

    with tc.tile_pool(name="persist", bufs=1) as persist:
        kT = persist.tile([128, QK // 128, T], BF, tag="kT")
        vaug = persist.tile([128, T // 128, H, DKH + 1], BF, tag="vaug")
        qT = persist.tile([128, QK // 128, TQ], BF, tag="qT")
        aoT = persist.tile([128, QK // 128, TQ], BF, tag="aoT")
        nc.vector.memset(vaug[:, :, :, DKH:DKH + 1], 1.0)

        # ---- phase 1: k/v projections (local half under SCHEME_C) ----
        with (
            nc.named_scope("p1_kvproj"),
            tc.tile_pool(name="wkv", bufs=1) as wkv_pool,
            tc.tile_pool(name="xk", bufs=17) as xk_pool,
            tc.tile_pool(name="xv", bufs=17) as xv_pool,
            tc.tile_pool(name="stg", bufs=6) as stg_pool,
            tc.tile_pool(name="ps1", bufs=6, space="PSUM") as ps1,
        ):
            wk_t = [wkv_pool.tile([128, QK], BF, tag=f"wk{k}", name=f"wk{k}")
                    for k in range(KD)]
            wv_t = [wkv_pool.tile([128, QK], BF, tag=f"wv{k}", name=f"wv{k}")
                    for k in range(KD)]
            ccin_f = cc_in[0, :] if SCHEME_C else None

            for nb in range(n_halves * TQ // 512):
                xk_t = []
                xv_t = []
                for k in range(KD):
                    xkt = xk_pool.tile([128, 512], BF, tag="xk")
                    xvt = xv_pool.tile([128, 512], BF, tag="xv")
                    # interleave weight-chunk and x-chunk loads so the first
                    # matmuls' inputs arrive first
                    if nb == 0:
                        nc.sync.dma_start(out=wk_t[k], in_=wk[k * 128:(k + 1) * 128, :])
                        nc.sync.dma_start(out=wv_t[k], in_=wv[k * 128:(k + 1) * 128, :])
                    nc.sync.dma_start(out=xkt, in_=xkT[k * 128:(k + 1) * 128, nb * 512:(nb + 1) * 512])
                    nc.sync.dma_start(out=xvt, in_=xvT[k * 128:(k + 1) * 128, nb * 512:(nb + 1) * 512])
                    xk_t.append(xkt)
                    xv_t.append(xvt)
                # kT[m-slice, this T block] = wk_slice.T @ xk
                for m in range(QK // 128):  # 8
                    ps = ps1.tile([128, 512], FP32, tag="ps1")
                    for k in range(KD):
                        nc.tensor.matmul(
                            ps, wk_t[k][:, m * 128:(m + 1) * 128], xk_t[k],
                            start=(k == 0), stop=(k == KD - 1))
                    if SCHEME_C:
                        st = stg_pool.tile([128, 512], BF, tag="stg")
                        nc.vector.tensor_copy(out=st, in_=ps)
                        dst = bass.AP(
                            tensor=ccin_f.tensor,
                            offset=m * 1024 + nb * 512,
                            ap=[[QK // 128 * TQ, 128], [1, 512]])
                        nc.sync.dma_start(out=dst, in_=st)
                    else:
                        nc.vector.tensor_copy(out=kT[:, m, nb * 512:(nb + 1) * 512], in_=ps)
                # v[T-row slice, V cols] = xv_slice.T @ wv
                for msl in range(4):
                    ms = nb * 4 + msl
                    for n in range(QK // 512):  # 2
                        ps = ps1.tile([128, 512], FP32, tag="ps1")
                        for k in range(KD):
                            nc.tensor.matmul(
                                ps, xv_t[k][:, msl * 128:(msl + 1) * 128],
                                wv_t[k][:, n * 512:(n + 1) * 512],
                                start=(k == 0), stop=(k == KD - 1))
                        if SCHEME_C:
                            st = stg_pool.tile([128, 512], BF, tag="stg")
                            nc.vector.tensor_copy(out=st, in_=ps)
                            dst = bass.AP(
                                tensor=ccin_f.tensor,
                                offset=KN + ms * 1024 + n * 512,
                                ap=[[TQ // 128 * QK, 128], [1, 512]])
                            nc.sync.dma_start(out=dst, in_=st)
                        else:
                            nc.vector.tensor_copy(
                                out=vaug[:, ms, n * 8:(n + 1) * 8, 0:DKH],
                                in_=ps.rearrange("p (h d) -> p h d", d=DKH))

                if nb == n_halves * TQ // 512 - 1:
                    # keep the PE array busy across the phase transition:
                    # a >3.4us idle gap lets the HAM re-throttle the clock
                    # to 1.2GHz for the next ~30us window. These filler
                    # matmuls read the last x tiles (so they schedule at
                    # the phase tail) and their results are never read.
                    for i in range(24):
                        ps = ps1.tile([128, 512], FP32, tag="ps1")
                        nc.tensor.matmul(
                            ps, xv_t[i % KD][:, 0:128], xk_t[(i + 1) % KD],
                            start=True, stop=True)

            if SCHEME_C:
                nc.gpsimd.collective_compute(
                    "AllGather", mybir.AluOpType.bypass,
                    replica_groups=[[0, 1], [2, 3], [4, 5], [6, 7]],
                    ins=[cc_in], outs=[cc_out])

        # ---- phase 2: q projection + attention ----
        with (
            nc.named_scope("p2_attn"),
            tc.tile_pool(name="wqp", bufs=1) as wq_pool,
            tc.tile_pool(name="xq", bufs=1) as xq_pool,
            tc.tile_pool(name="pt", bufs=12) as pt_pool,
            tc.tile_pool(name="dv", bufs=3) as dv_pool,
            tc.tile_pool(name="dsc", bufs=4, space="DRAM") as dr_pool,
            tc.tile_pool(name="pss", bufs=3, space="PSUM") as ps_s,
            tc.tile_pool(name="pav", bufs=2, space="PSUM") as ps_av,
        ):
            wq_t = [wq_pool.tile([128, QK], BF, tag=f"wq{k}", name=f"wq{k}")
                    for k in range(KD)]
            xq_t2 = [xq_pool.tile([128, TQ], BF, tag=f"xq{k}", name=f"xq{k}")
                     for k in range(KD)]
            for k in range(KD):
                nc.sync.dma_start(out=wq_t[k], in_=wq[k * 128:(k + 1) * 128, :])
                nc.sync.dma_start(out=xq_t2[k], in_=xqT[k * 128:(k + 1) * 128, :])

            if SCHEME_C:
                # scatter the gathered halves (rank order = T order) into
                # kT and vaug; overlaps with the q-projection below
                for r in range(2):
                    src_k = bass.AP(
                        tensor=cc_out.tensor,
                        offset=r * CCN,
                        ap=[[QK // 128 * TQ, 128], [TQ, QK // 128], [1, TQ]])
                    nc.sync.dma_start(out=kT[:, :, r * TQ:(r + 1) * TQ], in_=src_k)
                    for msl in range(TQ // 128):  # 8
                        ms = r * (TQ // 128) + msl
                        src_v = bass.AP(
                            tensor=cc_out.tensor,
                            offset=r * CCN + KN + msl * QK,
                            ap=[[TQ // 128 * QK, 128], [DKH, H], [1, DKH]])
                        nc.sync.dma_start(out=vaug[:, ms, :, 0:DKH], in_=src_v)

            for g in range(QK // 128):  # 8 head pairs
                for n in range(TQ // 512):  # 2
                    ps = ps_s.tile([128, 1024], FP32, tag="pss", name="psq")
                    for k in range(KD):
                        nc.tensor.matmul(
                            ps[:, 0:512], wq_t[k][:, g * 128:(g + 1) * 128],
                            xq_t2[k][:, n * 512:(n + 1) * 512],
                            start=(k == 0), stop=(k == KD - 1))
                    nc.vector.tensor_copy(out=qT[:, g, n * 512:(n + 1) * 512], in_=ps[:, 0:512])

                for hp in range(2):
                    h = 2 * g + hp
                    pk = slice(hp * 64, (hp + 1) * 64)
                    for tqb in range(TQ // 512):  # 2
                        qs = qT[pk, g, tqb * 512:(tqb + 1) * 512]
                        pts = []
                        for t in range(8):  # pairs of Tk chunks
                            pss = ps_s.tile([128, 1024], FP32, tag="pss")
                            for c2 in range(2):
                                c = 2 * t + c2
                                nc.tensor.matmul(
                                    pss[:, c2 * 512:(c2 + 1) * 512],
                                    kT[pk, g, c * 128:(c + 1) * 128],
                                    qs, start=True, stop=True)
                            ptt = pt_pool.tile([128, 1024], BF, tag="pt")
                            nc.scalar.activation(out=ptt, in_=pss, func=exp_f, scale=0.125)
                            pts.append(ptt)
                        pav = ps_av.tile([DKH + 1, 512], FP32, tag="pav")
                        for c in range(T // 128):  # 16
                            nc.tensor.matmul(
                                pav, vaug[:, c, h, :],
                                pts[c // 2][:, (c % 2) * 512:(c % 2 + 1) * 512],
                                start=(c == 0), stop=(c == T // 128 - 1))
                        linv = dv_pool.tile([1, 512], FP32, tag="linv")
                        nc.vector.reciprocal(out=linv, in_=pav[DKH:DKH + 1, :])
                        ldr = dr_pool.tile([1, 512], FP32, tag="ldr")
                        nc.gpsimd.dma_start(out=ldr, in_=linv)
                        lbc = dv_pool.tile([DKH, 512], FP32, tag="lbc")
                        nc.gpsimd.dma_start(out=lbc, in_=ldr.to_broadcast([DKH, 512]))
                        # pre-copy on DVE so the 2-input mul carries only a
                        # DVE-local wait (TensorTensor ISA allows one wait)
                        lbcc = dv_pool.tile([DKH, 512], FP32, tag="lbcc")
                        nc.vector.tensor_copy(out=lbcc, in_=lbc)
                        att = dv_pool.tile([DKH, 512], BF, tag="att")
                        nc.vector.tensor_mul(out=att, in0=pav[0:DKH, :], in1=lbcc)
                        nc.sync.dma_start(
                            out=aoT[pk, g, tqb * 512:(tqb + 1) * 512], in_=att)

        # ---- phase 3: output projection ----
        with (
            nc.named_scope("p3_oproj"),
            tc.tile_pool(name="wo", bufs=10) as wo_pool,
            tc.tile_pool(name="ostg", bufs=6) as o_pool,
            tc.tile_pool(name="pso", bufs=6, space="PSUM") as ps_o,
        ):
            KO = QK // 128  # 8
            # PE warmth bridge for the p2->p3 transition (see phase 1 tail):
            # reads aoT slice 6 (written near the end of attention) so the
            # scheduler places these in the gap before the first real MMs.
            for i in range(20):
                ps = ps_o.tile([128, 512], FP32, tag="pso")
                nc.tensor.matmul(
                    ps, aoT[:, 6, i * 128 % TQ:(i * 128 % TQ) + 128],
                    aoT[:, 6, 0:512], start=True, stop=True)
            for nb in range(D // 512):  # 4
                wo_t = []
                for k in range(KO):
                    wot = wo_pool.tile([128, 512], BF, tag="wo")
                    nc.sync.dma_start(out=wot, in_=wo[k * 128:(k + 1) * 128, nb * 512:(nb + 1) * 512])
                    wo_t.append(wot)
                for m in range(TQ // 128):  # 8
                    ps = ps_o.tile([128, 512], FP32, tag="pso")
                    for k in range(KO):
                        nc.tensor.matmul(
                            ps, aoT[:, k, m * 128:(m + 1) * 128], wo_t[k],
                            start=(k == 0), stop=(k == KO - 1))
                    stg = o_pool.tile([128, 512], FP32, tag="ostg")
                    nc.vector.tensor_copy(out=stg, in_=ps)
                    nc.sync.dma_start(
                        out=out[m * 128:(m + 1) * 128, nb * 512:(nb + 1) * 512], in_=stg)


def _build():
    if "nc" in _cache:
        return _cache["nc"]
    nc = bacc.Bacc("TRN2", target_bir_lowering=False, debug=False, num_devices=NCORES)
    xhalf = TQ if SCHEME_C else T
    xqT = nc.dram_tensor("xqT", [D, TQ], BF, kind="ExternalInput").ap()
    xkT = nc.dram_tensor("xkT", [D, xhalf], BF, kind="ExternalInput").ap()
    xvT = nc.dram_tensor("xvT", [D, xhalf], BF, kind="ExternalInput").ap()
    wq = nc.dram_tensor("wq", [D, QK], BF, kind="ExternalInput").ap()
    wk = nc.dram_tensor("wk", [D, QK], BF, kind="ExternalInput").ap()
    wv = nc.dram_tensor("wv", [D, QK], BF, kind="ExternalInput").ap()
    wo = nc.dram_tensor("wo", [QK, D], BF, kind="ExternalInput").ap()
    out = nc.dram_tensor("out", [TQ, D], mybir.dt.float32, kind="ExternalOutput").ap()
    cc_in = cc_out = None
    if SCHEME_C:
        cc_in = nc.dram_tensor("cc_in", [1, CCN], BF, kind="Internal").ap()
        cc_out = nc.dram_tensor("cc_out", [2, CCN], BF, kind="Internal").ap()
    with tile.TileContext(nc) as tc:
        _emit(tc, xqT, xkT, xvT, wq, wk, wv, wo, out, cc_in, cc_out)
    nc.compile()
    _cache["nc"] = nc
    return nc


def kernel(**inputs):
    global LAST
    Q = np.asarray(inputs["Q"], dtype=np.float32)
    K = np.asarray(inputs["K"], dtype=np.float32)
    V = np.asarray(inputs["V"], dtype=np.float32)
    wq_b = np.asarray(inputs["Wq"], dtype=np.float32).astype(BF16)
    wk_b = np.asarray(inputs["Wk"], dtype=np.float32).astype(BF16)
    wv_b = np.asarray(inputs["Wv"], dtype=np.float32).astype(BF16)
    wo_b = np.asarray(inputs["Wo"], dtype=np.float32).astype(BF16)

    nc = _build()
    in_maps = []
    for core in range(NCORES):
        b, s = core // 2, core % 2
        if SCHEME_C:
            xk = np.ascontiguousarray(K[b, s * TQ:(s + 1) * TQ, :].T).astype(BF16)
            xv = np.ascontiguousarray(V[b, s * TQ:(s + 1) * TQ, :].T).astype(BF16)
        else:
            xk = np.ascontiguousarray(K[b].T).astype(BF16)
            xv = np.ascontiguousarray(V[b].T).astype(BF16)
        in_maps.append({
            "xqT": np.ascontiguousarray(Q[b, s * TQ:(s + 1) * TQ, :].T).astype(BF16),
            "xkT": xk,
            "xvT": xv,
            "wq": wq_b, "wk": wk_b, "wv": wv_b, "wo": wo_b,
        })
    want_trace = bool(os.environ.get("BASS_TRACE"))
    if want_trace:
        _install_ntff_shim()
        try:
            res = run_bass_kernel_spmd(
                nc, in_maps, core_ids=list(range(NCORES)), trace=True)
        except Exception as e:  # profiling infra missing -> still get results
            print(f"trace run failed ({type(e).__name__}: {e}); retrying untraced")
            res = run_bass_kernel_spmd(nc, in_maps, core_ids=list(range(NCORES)))
    else:
        res = run_bass_kernel_spmd(nc, in_maps, core_ids=list(range(NCORES)))
    LAST = res
    if res.exec_time_ns is not None:
        print(f"HW exec time: {res.exec_time_ns} ns")

    out = np.empty((B, T, D), np.float32)
    for core in range(NCORES):
        b, s = core // 2, core % 2
        out[b, s * TQ:(s + 1) * TQ, :] = res.results[core]["out"]
    return out



# revision 15
# speedup vs baseline: 1.0760x; 1.0760x over previous
"""Multi-head attention on 8 Trainium2 NeuronCores (Bass/Tile).

Problem: B=4, T=2048, DIM=2048, H=16 heads, dk=dv=64.
  q = Q@Wq, k = K@Wk, v = V@Wv  (per head slices)
  out = softmax(q k^T / sqrt(dk)) v @ Wo

Sharding: data-parallel over batch (4) x query-row halves (2) = 8 cores.
Core (b, s) computes output rows [s*1024:(s+1)*1024] of batch b.
Each core projects k/v for its OWN T-half; the pair exchanges projected
k/v via a 2-rank AllGather (SCHEME_C). Attention + output projection are
core-local. With SCHEME_C=False each core recomputes the partner's k/v
projections instead (no collective).

Device layouts (bf16 compute, fp32 PSUM accumulation):
  xqT/xkT/xvT [D, TQ] = host-transposed input halves (D = contraction dim
    on partitions), wq/wk/wv [D, QK], wo [QK, D] natural (lhsT-ready)
  kT [QK, T]: head h rows 64h..64h+63 -> S^T matmul lhsT
  vaug [T, H, 65]: per head 64 v-cols + ones column (-> softmax row sums)
  S^T tile [Tk-chunk 128, Tq 512] = kT-chunk.T @ qT block (K=dk=64)
  P^T = exp(S^T/8)  (scores bounded ~+-5 -> no max-subtraction pass)
  aoT_aug [65, Tq] per head = vaug.T @ P^T accumulated over Tk chunks;
    row 64 = denominators l; rows/l via DRAM-bounce broadcast of 1/l
  out rows = aoT.T @ Wo accumulated over QK chunks.
"""

import os

import ml_dtypes
import numpy as np

import concourse.bass as bass
from concourse import bacc
import concourse.mybir as mybir
import concourse.tile as tile
from concourse.bass_utils import run_bass_kernel_spmd

BF16 = ml_dtypes.bfloat16
BF = mybir.dt.bfloat16
FP32 = mybir.dt.float32

B = 4
T = 2048
D = 2048
H = 16
DKH = 64
QK = H * DKH   # 1024
TQ = T // 2    # per-core query rows / local T half
KD = D // 128  # 16 contraction chunks for projections
NCORES = 8
SCHEME_C = False  # 2-rank AllGather measured slower than recomputing partner k/v

KN = 128 * (QK // 128) * TQ   # bf16 elems of one kT half
VN = 128 * (TQ // 128) * QK   # bf16 elems of one v half
CCN = KN + VN

LAST = None  # BassKernelResults of the most recent run (for test harness)

_cache = {}


def _install_ntff_shim():
    """Provide antenv.axon_hooks + disable artifact upload so that
    run_bass_kernel_spmd(trace=True) can profile under axon in this image."""
    import sys
    import types

    try:
        import antenv.axon_hooks  # noqa: F401
    except ImportError:
        import antenv
        mod = types.ModuleType("antenv.axon_hooks")
        _h = [None]
        mod.set_axon_ntff_profile_hook = lambda h: _h.__setitem__(0, h)
        mod.get_axon_ntff_profile_hook = lambda: _h[0]
        sys.modules["antenv.axon_hooks"] = mod
        antenv.axon_hooks = mod
        try:
            from trn_agent_boot.trn_boot import _ntff_profile_via_ctypes
            mod.set_axon_ntff_profile_hook(
                _ntff_profile_via_ctypes("/opt/axon/libaxon_pjrt.so"))
        except Exception as e:
            print(f"ntff hook registration failed: {e}")
    try:
        import concourse.bass_utils as bu
        bu.upload_artifacts = lambda tmpdir: f"local:{tmpdir}"
    except Exception:
        pass


def _emit(tc, xqT, xkT, xvT, wq, wk, wv, wo, out, cc_in, cc_out):
    nc = tc.nc
    exp_f = mybir.ActivationFunctionType.Exp
    n_halves = 1 if SCHEME_C else 2

    with tc.tile_pool(name="persist", bufs=1) as persist:
        kT = persist.tile([128, QK // 128, T], BF, tag="kT")
        vaug = persist.tile([128, T // 128, H, DKH + 1], BF, tag="vaug")
        qT = persist.tile([128, QK // 128, TQ], BF, tag="qT")
        aoT = persist.tile([128, QK // 128, TQ], BF, tag="aoT")
        nc.vector.memset(vaug[:, :, :, DKH:DKH + 1], 1.0)

        # ---- phase 1: k/v projections (local half under SCHEME_C) ----
        with (
            nc.named_scope("p1_kvproj"),
            tc.tile_pool(name="wkv", bufs=1) as wkv_pool,
            tc.tile_pool(name="xk", bufs=17) as xk_pool,
            tc.tile_pool(name="xv", bufs=17) as xv_pool,
            tc.tile_pool(name="stg", bufs=6) as stg_pool,
            tc.tile_pool(name="ps1", bufs=6, space="PSUM") as ps1,
        ):
            wk_t = [wkv_pool.tile([128, QK], BF, tag=f"wk{k}", name=f"wk{k}")
                    for k in range(KD)]
            wv_t = [wkv_pool.tile([128, QK], BF, tag=f"wv{k}", name=f"wv{k}")
                    for k in range(KD)]
            ccin_f = cc_in[0, :] if SCHEME_C else None

            for nb in range(n_halves * TQ // 512):
                xk_t = []
                xv_t = []
                for k in range(KD):
                    xkt = xk_pool.tile([128, 512], BF, tag="xk")
                    xvt = xv_pool.tile([128, 512], BF, tag="xv")
                    # interleave weight-chunk and x-chunk loads so the first
                    # matmuls' inputs arrive first
                    if nb == 0:
                        nc.sync.dma_start(out=wk_t[k], in_=wk[k * 128:(k + 1) * 128, :])
                        nc.sync.dma_start(out=wv_t[k], in_=wv[k * 128:(k + 1) * 128, :])
                    nc.sync.dma_start(out=xkt, in_=xkT[k * 128:(k + 1) * 128, nb * 512:(nb + 1) * 512])
                    nc.sync.dma_start(out=xvt, in_=xvT[k * 128:(k + 1) * 128, nb * 512:(nb + 1) * 512])
                    xk_t.append(xkt)
                    xv_t.append(xvt)
                # kT[m-slice, this T block] = wk_slice.T @ xk
                for m in range(QK // 128):  # 8
                    ps = ps1.tile([128, 512], FP32, tag="ps1")
                    for k in range(KD):
                        nc.tensor.matmul(
                            ps, wk_t[k][:, m * 128:(m + 1) * 128], xk_t[k],
                            start=(k == 0), stop=(k == KD - 1))
                    if SCHEME_C:
                        st = stg_pool.tile([128, 512], BF, tag="stg")
                        nc.vector.tensor_copy(out=st, in_=ps)
                        dst = bass.AP(
                            tensor=ccin_f.tensor,
                            offset=m * 1024 + nb * 512,
                            ap=[[QK // 128 * TQ, 128], [1, 512]])
                        nc.sync.dma_start(out=dst, in_=st)
                    else:
                        nc.vector.tensor_copy(out=kT[:, m, nb * 512:(nb + 1) * 512], in_=ps)
                # v[T-row slice, V cols] = xv_slice.T @ wv
                for msl in range(4):
                    ms = nb * 4 + msl
                    for n in range(QK // 512):  # 2
                        ps = ps1.tile([128, 512], FP32, tag="ps1")
                        for k in range(KD):
                            nc.tensor.matmul(
                                ps, xv_t[k][:, msl * 128:(msl + 1) * 128],
                                wv_t[k][:, n * 512:(n + 1) * 512],
                                start=(k == 0), stop=(k == KD - 1))
                        if SCHEME_C:
                            st = stg_pool.tile([128, 512], BF, tag="stg")
                            nc.vector.tensor_copy(out=st, in_=ps)
                            dst = bass.AP(
                                tensor=ccin_f.tensor,
                                offset=KN + ms * 1024 + n * 512,
                                ap=[[TQ // 128 * QK, 128], [1, 512]])
                            nc.sync.dma_start(out=dst, in_=st)
                        else:
                            nc.vector.tensor_copy(
                                out=vaug[:, ms, n * 8:(n + 1) * 8, 0:DKH],
                                in_=ps.rearrange("p (h d) -> p h d", d=DKH))

                if nb == n_halves * TQ // 512 - 1:
                    # keep the PE array busy across the phase transition:
                    # a >3.4us idle gap lets the HAM re-throttle the clock
                    # to 1.2GHz for the next ~30us window. These filler
                    # matmuls read the last x tiles (so they schedule at
                    # the phase tail) and their results are never read.
                    for i in range(24):
                        ps = ps1.tile([128, 512], FP32, tag="ps1")
                        nc.tensor.matmul(
                            ps, xv_t[i % KD][:, 0:128], xk_t[(i + 1) % KD],
                            start=True, stop=True)

            if SCHEME_C:
                nc.gpsimd.collective_compute(
                    "AllGather", mybir.AluOpType.bypass,
                    replica_groups=[[0, 1], [2, 3], [4, 5], [6, 7]],
                    ins=[cc_in], outs=[cc_out])

        # ---- phase 2: q projection + attention ----
        with (
            nc.named_scope("p2_attn"),
            tc.tile_pool(name="wqp", bufs=1) as wq_pool,
            tc.tile_pool(name="xq", bufs=1) as xq_pool,
            tc.tile_pool(name="pt", bufs=12) as pt_pool,
            tc.tile_pool(name="dv", bufs=3) as dv_pool,
            tc.tile_pool(name="dsc", bufs=4, space="DRAM") as dr_pool,
            tc.tile_pool(name="psq", bufs=2, space="PSUM") as ps_q,
            tc.tile_pool(name="pss", bufs=2, space="PSUM") as ps_s,
            tc.tile_pool(name="pav", bufs=2, space="PSUM") as ps_av,
        ):
            wq_t = [wq_pool.tile([128, QK], BF, tag=f"wq{k}", name=f"wq{k}")
                    for k in range(KD)]
            xq_t2 = [xq_pool.tile([128, TQ], BF, tag=f"xq{k}", name=f"xq{k}")
                     for k in range(KD)]
            for k in range(KD):
                nc.sync.dma_start(out=wq_t[k], in_=wq[k * 128:(k + 1) * 128, :])
                nc.sync.dma_start(out=xq_t2[k], in_=xqT[k * 128:(k + 1) * 128, :])

            if SCHEME_C:
                # scatter the gathered halves (rank order = T order) into
                # kT and vaug; overlaps with the q-projection below
                for r in range(2):
                    src_k = bass.AP(
                        tensor=cc_out.tensor,
                        offset=r * CCN,
                        ap=[[QK // 128 * TQ, 128], [TQ, QK // 128], [1, TQ]])
                    nc.sync.dma_start(out=kT[:, :, r * TQ:(r + 1) * TQ], in_=src_k)
                    for msl in range(TQ // 128):  # 8
                        ms = r * (TQ // 128) + msl
                        src_v = bass.AP(
                            tensor=cc_out.tensor,
                            offset=r * CCN + KN + msl * QK,
                            ap=[[TQ // 128 * QK, 128], [DKH, H], [1, DKH]])
                        nc.sync.dma_start(out=vaug[:, ms, :, 0:DKH], in_=src_v)

            for g in range(QK // 128):  # 8 head pairs
                for n in range(TQ // 512):  # 2
                    ps = ps_q.tile([128, 512], FP32, tag="psq", name="psq")
                    for k in range(KD):
                        nc.tensor.matmul(
                            ps, wq_t[k][:, g * 128:(g + 1) * 128],
                            xq_t2[k][:, n * 512:(n + 1) * 512],
                            start=(k == 0), stop=(k == KD - 1))
                    nc.vector.tensor_copy(out=qT[:, g, n * 512:(n + 1) * 512], in_=ps)

                for hp in range(2):
                    h = 2 * g + hp
                    pk = slice(hp * 64, (hp + 1) * 64)
                    for tqb in range(TQ // 512):  # 2
                        qs = qT[pk, g, tqb * 512:(tqb + 1) * 512]
                        pts = []
                        for t in range(8):  # pairs of Tk chunks
                            pss = ps_s.tile([128, 1024], FP32, tag="pss")
                            for c2 in range(2):
                                c = 2 * t + c2
                                nc.tensor.matmul(
                                    pss[:, c2 * 512:(c2 + 1) * 512],
                                    kT[pk, g, c * 128:(c + 1) * 128],
                                    qs, start=True, stop=True)
                            ptt = pt_pool.tile([128, 1024], BF, tag="pt")
                            nc.scalar.activation(out=ptt, in_=pss, func=exp_f, scale=0.125)
                            pts.append(ptt)
                        pav = ps_av.tile([DKH + 1, 512], FP32, tag="pav")
                        for c in range(T // 128):  # 16
                            nc.tensor.matmul(
                                pav, vaug[:, c, h, :],
                                pts[c // 2][:, (c % 2) * 512:(c % 2 + 1) * 512],
                                start=(c == 0), stop=(c == T // 128 - 1))
                        linv = dv_pool.tile([1, 512], FP32, tag="linv")
                        nc.vector.reciprocal(out=linv, in_=pav[DKH:DKH + 1, :])
                        ldr = dr_pool.tile([1, 512], FP32, tag="ldr")
                        nc.gpsimd.dma_start(out=ldr, in_=linv)
                        lbc = dv_pool.tile([DKH, 512], FP32, tag="lbc")
                        nc.gpsimd.dma_start(out=lbc, in_=ldr.to_broadcast([DKH, 512]))
                        # pre-copy on DVE so the 2-input mul carries only a
                        # DVE-local wait (TensorTensor ISA allows one wait)
                        lbcc = dv_pool.tile([DKH, 512], FP32, tag="lbcc")
                        nc.vector.tensor_copy(out=lbcc, in_=lbc)
                        att = dv_pool.tile([DKH, 512], BF, tag="att")
                        nc.vector.tensor_mul(out=att, in0=pav[0:DKH, :], in1=lbcc)
                        nc.sync.dma_start(
                            out=aoT[pk, g, tqb * 512:(tqb + 1) * 512], in_=att)

        # ---- phase 3: output projection ----
        with (
            nc.named_scope("p3_oproj"),
            tc.tile_pool(name="wo", bufs=10) as wo_pool,
            tc.tile_pool(name="ostg", bufs=6) as o_pool,
            tc.tile_pool(name="pso", bufs=6, space="PSUM") as ps_o,
        ):
            KO = QK // 128  # 8
            # PE warmth bridge for the p2->p3 transition (see phase 1 tail):
            # reads aoT slice 6 (written near the end of attention) so the
            # scheduler places these in the gap before the first real MMs.
            for i in range(20):
                ps = ps_o.tile([128, 512], FP32, tag="pso")
                nc.tensor.matmul(
                    ps, aoT[:, 6, i * 128 % TQ:(i * 128 % TQ) + 128],
                    aoT[:, 6, 0:512], start=True, stop=True)
            for nb in range(D // 512):  # 4
                wo_t = []
                for k in range(KO):
                    wot = wo_pool.tile([128, 512], BF, tag="wo")
                    nc.sync.dma_start(out=wot, in_=wo[k * 128:(k + 1) * 128, nb * 512:(nb + 1) * 512])
                    wo_t.append(wot)
                for m in range(TQ // 128):  # 8
                    ps = ps_o.tile([128, 512], FP32, tag="pso")
                    for k in range(KO):
                        nc.tensor.matmul(
                            ps, aoT[:, k, m * 128:(m + 1) * 128], wo_t[k],
                            start=(k == 0), stop=(k == KO - 1))
                    stg = o_pool.tile([128, 512], FP32, tag="ostg")
                    nc.vector.tensor_copy(out=stg, in_=ps)
                    nc.sync.dma_start(
                        out=out[m * 128:(m + 1) * 128, nb * 512:(nb + 1) * 512], in_=stg)


def _build():
    if "nc" in _cache:
        return _cache["nc"]
    nc = bacc.Bacc("TRN2", target_bir_lowering=False, debug=False, num_devices=NCORES)
    xhalf = TQ if SCHEME_C else T
    xqT = nc.dram_tensor("xqT", [D, TQ], BF, kind="ExternalInput").ap()
    xkT = nc.dram_tensor("xkT", [D, xhalf], BF, kind="ExternalInput").ap()
    xvT = nc.dram_tensor("xvT", [D, xhalf], BF, kind="ExternalInput").ap()
    wq = nc.dram_tensor("wq", [D, QK], BF, kind="ExternalInput").ap()
    wk = nc.dram_tensor("wk", [D, QK], BF, kind="ExternalInput").ap()
    wv = nc.dram_tensor("wv", [D, QK], BF, kind="ExternalInput").ap()
    wo = nc.dram_tensor("wo", [QK, D], BF, kind="ExternalInput").ap()
    out = nc.dram_tensor("out", [TQ, D], mybir.dt.float32, kind="ExternalOutput").ap()
    cc_in = cc_out = None
    if SCHEME_C:
        cc_in = nc.dram_tensor("cc_in", [1, CCN], BF, kind="Internal").ap()
        cc_out = nc.dram_tensor("cc_out", [2, CCN], BF, kind="Internal").ap()
    with tile.TileContext(nc) as tc:
        _emit(tc, xqT, xkT, xvT, wq, wk, wv, wo, out, cc_in, cc_out)
    nc.compile()
    _cache["nc"] = nc
    return nc


def kernel(**inputs):
    global LAST
    Q = np.asarray(inputs["Q"], dtype=np.float32)
    K = np.asarray(inputs["K"], dtype=np.float32)
    V = np.asarray(inputs["V"], dtype=np.float32)
    wq_b = np.asarray(inputs["Wq"], dtype=np.float32).astype(BF16)
    wk_b = np.asarray(inputs["Wk"], dtype=np.float32).astype(BF16)
    wv_b = np.asarray(inputs["Wv"], dtype=np.float32).astype(BF16)
    wo_b = np.asarray(inputs["Wo"], dtype=np.float32).astype(BF16)

    nc = _build()
    in_maps = []
    for core in range(NCORES):
        b, s = core // 2, core % 2
        if SCHEME_C:
            xk = np.ascontiguousarray(K[b, s * TQ:(s + 1) * TQ, :].T).astype(BF16)
            xv = np.ascontiguousarray(V[b, s * TQ:(s + 1) * TQ, :].T).astype(BF16)
        else:
            xk = np.ascontiguousarray(K[b].T).astype(BF16)
            xv = np.ascontiguousarray(V[b].T).astype(BF16)
        in_maps.append({
            "xqT": np.ascontiguousarray(Q[b, s * TQ:(s + 1) * TQ, :].T).astype(BF16),
            "xkT": xk,
            "xvT": xv,
            "wq": wq_b, "wk": wk_b, "wv": wv_b, "wo": wo_b,
        })
    want_trace = bool(os.environ.get("BASS_TRACE"))
    if want_trace:
        _install_ntff_shim()
        try:
            res = run_bass_kernel_spmd(
                nc, in_maps, core_ids=list(range(NCORES)), trace=True)
        except Exception as e:  # profiling infra missing -> still get results
            print(f"trace run failed ({type(e).__name__}: {e}); retrying untraced")
            res = run_bass_kernel_spmd(nc, in_maps, core_ids=list(range(NCORES)))
    else:
        res = run_bass_kernel_spmd(nc, in_maps, core_ids=list(range(NCORES)))
    LAST = res
    if res.exec_time_ns is not None:
        print(f"HW exec time: {res.exec_time_ns} ns")

    out = np.empty((B, T, D), np.float32)
    for core in range(NCORES):
        b, s = core // 2, core % 2
        out[b, s * TQ:(s + 1) * TQ, :] = res.results[core]["out"]
    return out

